# revision 3
# baseline (speedup 1.0000x reference)
"""GCN 2-layer message-passing block on 8 Trainium2 NeuronCores.

Math restructuring (validated against the jax reference to 7e-7 in fp64):
  dis = deg^-0.5 (deg over edge sources), x~ = dis * x (host)
  a[d] = sum_{e->d} dis[row_e], c = dis*a (host)
  g1[v]   = sum_{e: col=v} x~[row_e]            -- aggregation 1
  table1  = dis^2 * g1
  q[v]    = table1[v] @ W1.T + (dis^2*a)[v]*b1  -- Lin1 + rank-1 bias
  z2[d]   = sum_{e->d} q[row_e]                 -- aggregation 2
  y2      = (dis*z2) @ W2.T + c outer b2        -- Lin2 + rank-1 bias

Sharding: destination nodes split into 8 contiguous blocks of 12500.
Zero-communication: each core computes table1 at S_c (the distinct sources of
its own edges) by aggregating host-packed x~ edge tokens for ALL edges into
S_c.  Aggregations run as matmuls with static one-hot block matrices (S1/S2),
accumulated in fp32 PSUM.  Gathers use the custom dma_gather (transpose=True
-> feature-major blocks that feed the tensor engine directly as lhsT).
"""
import os
import sys

sys.path.insert(0, "/opt/trn_rl_repo")

import numpy as np
import ml_dtypes

BF16 = ml_dtypes.bfloat16

N_NODES = 100000
N_EDGES = 200000
H = 384
KB = H // 128          # 3 contraction blocks of 128
M_CORES = 8
NPC = N_NODES // M_CORES   # 12500
NTB = (NPC + 127) // 128   # 98 owned-dest tiles
NPC_PAD = NTB * 128        # 12544


def _pack_tokens(dest_local, ntiles, blocks_per_tile):
    """Sorted slot assignment: tokens sorted by dest tile; tile t's tokens at
    slots [t*blocks*128, ...). Returns (order, slot) for valid tokens."""
    order = np.argsort(dest_local // 128, kind="stable")
    tile_id = dest_local[order] // 128
    tile_start = np.searchsorted(tile_id, np.arange(ntiles))
    rank = np.arange(dest_local.size) - tile_start[tile_id]
    slot = tile_id * (blocks_per_tile * 128) + rank
    return order, slot


def _wrap_idx(idx):
    """int16 index array -> [128, n/16] layout: token j at [j%16, j//16],
    replicated across the 8 gpsimd q7 cores (partition groups of 16)."""
    n = idx.size
    assert n % 16 == 0
    w = idx.reshape(n // 16, 16).T.astype(np.int16)   # [16, n/16]
    return np.tile(w, (8, 1))                          # [128, n/16]


def _prep(x, edge_index, W1, b1, W2, b2):
    row = np.asarray(edge_index[0], dtype=np.int64)
    col = np.asarray(edge_index[1], dtype=np.int64)
    xf = np.asarray(x, dtype=np.float64)

    deg = np.bincount(row, minlength=N_NODES).astype(np.float64)
    dis = deg ** -0.5
    a = np.bincount(col, weights=dis[row], minlength=N_NODES)
    cvec = dis * a
    dis2a = dis * dis * a
    xt = (dis[:, None] * xf).astype(BF16)

    core_of = col // NPC

    # ---- pass 1: uniform sizes across cores ----
    metas = []
    NTA = 0
    B1 = 0
    B2 = 0
    for cc in range(M_CORES):
        em = core_of == cc
        er, ec = row[em], col[em]
        S = np.unique(er)
        pos = np.full(N_NODES, -1, dtype=np.int64)
        pos[S] = np.arange(S.size)
        d2_all = pos[col]
        e2m = d2_all >= 0
        d2, r2 = d2_all[e2m], row[e2m]
        nta = (S.size + 127) // 128
        cntA = np.bincount(d2 // 128, minlength=nta)
        cntB = np.bincount((ec - cc * NPC) // 128, minlength=NTB)
        NTA = max(NTA, nta)
        B1 = max(B1, int(-(-cntA.max() // 128)))
        B2 = max(B2, int(-(-cntB.max() // 128)))
        metas.append((er, ec, S, pos, d2, r2))

    n1 = NTA * B1 * 128
    n2 = NTB * B2 * 128

    w1t = np.ascontiguousarray(
        W1.T.astype(BF16).reshape(KB, 128, H).transpose(1, 0, 2))  # [128,KB,H]
    w2t = np.ascontiguousarray(
        W2.T.astype(BF16).reshape(KB, 128, H).transpose(1, 0, 2))
    b1r = np.asarray(b1, dtype=BF16).reshape(1, H)
    b2r = np.asarray(b2, dtype=BF16).reshape(1, H)

    in_maps = []
    for cc in range(M_CORES):
        er, ec, S, pos, d2, r2 = metas[cc]

        # stage A: aggregate x~ tokens into table1 rows (compact dests in S)
        orderA, slotA = _pack_tokens(d2, NTA, B1)
        t1 = np.zeros((n1, H), dtype=BF16)
        t1[slotA] = xt[r2[orderA]]
        s1 = np.zeros((NTA * B1, 128, 128), dtype=BF16)
        s1[slotA // 128, slotA % 128, d2[orderA] % 128] = 1.0

        tmp = np.zeros(NTA * 128, dtype=np.float32)
        tmp[: S.size] = (dis[S] ** 2).astype(np.float32)
        dis2arr = np.ascontiguousarray(tmp.reshape(NTA, 128).T)

        # stage B/C: own edges -> gather table1 rows, Lin1, aggregate to z2
        dl = ec - cc * NPC
        orderB, slotB = _pack_tokens(dl, NTB, B2)
        gidx = np.zeros(n2, dtype=np.int64)
        gidx[slotB] = pos[er[orderB]]
        brow = np.zeros((1, n2), dtype=BF16)
        brow[0, slotB] = dis2a[er[orderB]].astype(BF16)
        s2 = np.zeros((NTB * B2, 128, 128), dtype=BF16)
        s2[slotB // 128, slotB % 128, dl[orderB] % 128] = 1.0

        tmp = np.zeros(NPC_PAD, dtype=np.float32)
        tmp[:NPC] = dis[cc * NPC : (cc + 1) * NPC].astype(np.float32)
        disarr = np.ascontiguousarray(tmp.reshape(NTB, 128).T)

        crow = np.zeros((1, NPC_PAD), dtype=BF16)
        crow[0, :NPC] = cvec[cc * NPC : (cc + 1) * NPC].astype(BF16)

        in_maps.append({
            "t1": t1,
            "s1": s1,
            "dis2": dis2arr,
            "gidx": _wrap_idx(gidx),
            "brow": brow,
            "s2": s2,
            "disc": disarr,
            "iidx": _wrap_idx(np.arange(NPC_PAD)),
            "crow": crow,
            "w1t": w1t, "b1r": b1r, "w2t": w2t, "b2r": b2r,
        })
    return in_maps, NTA, B1, B2


def _build(NTA, B1, B2):
    import concourse.bass as bass
    import concourse.bacc as bacc
    import concourse.mybir as mybir
    import concourse.tile as tile

    dt = mybir.dt
    n1 = NTA * B1 * 128
    n2 = NTB * B2 * 128

    nc = bacc.Bacc(None, target_bir_lowering=False)
    t1 = nc.dram_tensor("t1", [n1, H], dt.bfloat16, kind="ExternalInput")
    s1 = nc.dram_tensor("s1", [NTA * B1, 128, 128], dt.bfloat16, kind="ExternalInput")
    dis2 = nc.dram_tensor("dis2", [128, NTA], dt.float32, kind="ExternalInput")
    gidx = nc.dram_tensor("gidx", [128, n2 // 16], dt.int16, kind="ExternalInput")
    brow = nc.dram_tensor("brow", [1, n2], dt.bfloat16, kind="ExternalInput")
    s2 = nc.dram_tensor("s2", [NTB * B2, 128, 128], dt.bfloat16, kind="ExternalInput")
    disc = nc.dram_tensor("disc", [128, NTB], dt.float32, kind="ExternalInput")
    iidx = nc.dram_tensor("iidx", [128, NPC_PAD // 16], dt.int16, kind="ExternalInput")
    crow = nc.dram_tensor("crow", [1, NPC_PAD], dt.bfloat16, kind="ExternalInput")
    w1t = nc.dram_tensor("w1t", [128, KB, H], dt.bfloat16, kind="ExternalInput")
    b1r = nc.dram_tensor("b1r", [1, H], dt.bfloat16, kind="ExternalInput")
    w2t = nc.dram_tensor("w2t", [128, KB, H], dt.bfloat16, kind="ExternalInput")
    b2r = nc.dram_tensor("b2r", [1, H], dt.bfloat16, kind="ExternalInput")
    tab1 = nc.dram_tensor("tab1", [NTA * 128, H], dt.bfloat16, kind="Internal")
    ztab = nc.dram_tensor("ztab", [NPC_PAD, H], dt.bfloat16, kind="Internal")
    out = nc.dram_tensor("out", [NPC_PAD, H], dt.float32, kind="ExternalOutput")

    with tile.TileContext(nc) as tc:
        with (
            tc.tile_pool(name="const", bufs=1) as cp,
            tc.tile_pool(name="io", bufs=3) as iop,
            tc.tile_pool(name="ps", bufs=2, space="PSUM") as psp,
        ):
            w1t_sb = cp.tile([128, KB, H], dt.bfloat16)
            nc.sync.dma_start(w1t_sb[:], w1t[:])
            w2t_sb = cp.tile([128, KB, H], dt.bfloat16)
            nc.sync.dma_start(w2t_sb[:], w2t[:])
            b1r_sb = cp.tile([1, H], dt.bfloat16)
            nc.sync.dma_start(b1r_sb[:], b1r[:])
            b2r_sb = cp.tile([1, H], dt.bfloat16)
            nc.sync.dma_start(b2r_sb[:], b2r[:])
            dis2_sb = cp.tile([128, NTA], dt.float32)
            nc.sync.dma_start(dis2_sb[:], dis2[:])
            disc_sb = cp.tile([128, NTB], dt.float32)
            nc.sync.dma_start(disc_sb[:], disc[:])
            gidx_sb = cp.tile([128, n2 // 16], dt.int16)
            nc.sync.dma_start(gidx_sb[:], gidx[:])
            iidx_sb = cp.tile([128, NPC_PAD // 16], dt.int16)
            nc.sync.dma_start(iidx_sb[:], iidx[:])
            brow_sb = cp.tile([1, n2], dt.bfloat16)
            nc.sync.dma_start(brow_sb[:], brow[:])
            crow_sb = cp.tile([1, NPC_PAD], dt.bfloat16)
            nc.sync.dma_start(crow_sb[:], crow[:])

            # ---------- stage A: table1 = dis^2 * (S1 @ t1) ----------
            for i in range(NTA):
                psA = psp.tile([128, H], dt.float32, tag="psA")
                for b in range(B1):
                    blk = i * B1 + b
                    t1_sb = iop.tile([128, H], dt.bfloat16, tag="t1")
                    nc.sync.dma_start(t1_sb[:], t1[blk * 128:(blk + 1) * 128, :])
                    s1_sb = iop.tile([128, 128], dt.bfloat16, tag="s1")
                    nc.sync.dma_start(s1_sb[:], s1[blk])
                    nc.tensor.matmul(psA[:], s1_sb[:], t1_sb[:],
                                     start=(b == 0), stop=(b == B1 - 1))
                tw = iop.tile([128, H], dt.bfloat16, tag="tw")
                nc.vector.tensor_scalar_mul(tw[:], psA[:], dis2_sb[:, i:i + 1])
                nc.sync.dma_start(tab1[i * 128:(i + 1) * 128, :], tw[:])

            # ---------- stage B/C: z2 tiles ----------
            for j in range(NTB):
                g_sb = iop.tile([128, KB, B2 * 128], dt.bfloat16, tag="g")
                nc.gpsimd.dma_gather(
                    g_sb[:], tab1[:, :],
                    gidx_sb[:, j * (B2 * 8):(j + 1) * (B2 * 8)],
                    B2 * 128, B2 * 128, H, transpose=True)
                u1 = iop.tile([128, B2, H], dt.bfloat16, tag="u1")
                for b in range(B2):
                    psB = psp.tile([128, H], dt.float32, tag="psB")
                    for k in range(KB):
                        nc.tensor.matmul(psB[:], g_sb[:, k, b * 128:(b + 1) * 128],
                                         w1t_sb[:, k, :], start=(k == 0), stop=False)
                    s0 = (j * B2 + b) * 128
                    nc.tensor.matmul(psB[:], brow_sb[:, s0:s0 + 128], b1r_sb[:],
                                     start=False, stop=True)
                    nc.vector.tensor_copy(u1[:, b, :], psB[:])
                psC = psp.tile([128, H], dt.float32, tag="psC")
                for b in range(B2):
                    blk = j * B2 + b
                    s2_sb = iop.tile([128, 128], dt.bfloat16, tag="s2")
                    nc.sync.dma_start(s2_sb[:], s2[blk])
                    nc.tensor.matmul(psC[:], s2_sb[:], u1[:, b, :],
                                     start=(b == 0), stop=(b == B2 - 1))
                zw = iop.tile([128, H], dt.bfloat16, tag="zw")
                nc.vector.tensor_scalar_mul(zw[:], psC[:], disc_sb[:, j:j + 1])
                nc.sync.dma_start(ztab[j * 128:(j + 1) * 128, :], zw[:])

            # ---------- stage D: out = (ztab gathered^T) @ W2T + c x b2 ----------
            for j in range(NTB):
                zg = iop.tile([128, KB, 128], dt.bfloat16, tag="zg")
                nc.gpsimd.dma_gather(
                    zg[:], ztab[:, :], iidx_sb[:, j * 8:(j + 1) * 8],
                    128, 128, H, transpose=True)
                psD = psp.tile([128, H], dt.float32, tag="psD")
                for k in range(KB):
                    nc.tensor.matmul(psD[:], zg[:, k, :], w2t_sb[:, k, :],
                                     start=(k == 0), stop=False)
                nc.tensor.matmul(psD[:], crow_sb[:, j * 128:(j + 1) * 128],
                                 b2r_sb[:], start=False, stop=True)
                ow = iop.tile([128, H], dt.float32, tag="ow")
                nc.vector.tensor_copy(ow[:], psD[:])
                nc.sync.dma_start(out[j * 128:(j + 1) * 128, :], ow[:])

    nc.compile()
    return nc


_CACHE = {}


def kernel(x, edge_index, W1, b1, W2, b2):
    from concourse import bass_utils

    in_maps, NTA, B1, B2 = _prep(x, edge_index, W1, b1, W2, b2)
    key = (NTA, B1, B2)
    if key not in _CACHE:
        _CACHE[key] = _build(NTA, B1, B2)
    nc = _CACHE[key]
    res = bass_utils.run_bass_kernel_spmd(nc, in_maps, core_ids=list(range(M_CORES)))
    outs = [np.asarray(res.results[i]["out"][:NPC]) for i in range(M_CORES)]
    return np.concatenate(outs, axis=0).astype(np.float32)
